# revision 29
# baseline (speedup 1.0000x reference)
"""Trainium2 Bass kernel for nn_MAE_65025804861607 (MAE block: fused
qkv/dwconv/fc/depconv branch + 4-direction GroupMamba selective scan).

Data-parallel over batch: 16 images -> 8 cores x 2 images. v3 layout:
  conv branch: f2 = sum_tap (FCbd . diag(dw_tap) . Wqkv) @ x_shift_tap + fc_b
               out_conv = sum_tap BDdep_tap @ f2_shift_tap + dep_b
    f2 taps are PAIRED: x_pad tiles hold the padded image on partitions 0-63
    and a 1-column-left-shifted copy on 64-127, so taps (ty,0)+(ty,1) run as
    one 128-contraction matmul (6 passes instead of 9).
  mamba branch: LN stats stay in ROW form (no col-form transpose DMAs); the
    normalized xn is materialized once so projections/Dp-skip need no rank-1
    mean corrections; scan lanes are (d*8+n) so dt/u/B/C replication runs as
    direct SBUF->SBUF broadcast DMAs (no DRAM round trip); the 4 raster
    directions are pure access patterns on the scan instruction (the
    column-major groups scan through transposed APs; nothing is ever
    re-materialized in scan order).

kernel() compiles once per reps value and caches the jitted PJRT executable.
"""
import sys
import numpy as np

sys.path.insert(0, '/opt/trn_rl_repo')

import concourse.bass as bass
import concourse.mybir as mybir
from concourse.tile import TileContext
from concourse.bass_utils import run_bass_kernel_spmd

F32 = mybir.dt.float32
BF16 = mybir.dt.bfloat16
AF = mybir.ActivationFunctionType
OP = mybir.AluOpType

NCORES = 8
IPC = 2               # images per core
C = 64
H = W = 64
L = H * W             # 4096
NG, DG, DSTATE = 4, 16, 8
Hp, Wp = H + 2, W + 2
PADL = Hp * Wp        # 4356
TC = 512              # psum chunk = 8 image rows
NCH = L // TC         # 8

_CACHE = {}
DEBUG_SKIP = set()


# ----------------------------------------------------------------------------
# Walrus here allows only 1 embedded sem-wait per instruction (2 on
# EventSemaphore). Hoist excess waits into standalone EventSemaphores.
# ----------------------------------------------------------------------------
def _fix_waits_json(data):
    lim = {"EventSemaphore": 2}
    for fn in data.get("functions", []):
        for blk in fn.get("blocks", []):
            out = []
            for ins in blk.get("instructions", []):
                si = ins.get("sync_info")
                ow = (si or {}).get("on_wait") or []
                limit = lim.get(ins.get("opcode"), 1)
                if len(ow) > limit:
                    excess = ow[: len(ow) - limit]
                    si["on_wait"] = ow[len(ow) - limit:]
                    for k, wv in enumerate(excess):
                        out.append({
                            "debug": ins.get("debug", 0),
                            "engine": ins["engine"],
                            "ins": [], "outs": [],
                            "name": f"{ins['name']}_xw{k}",
                            "opcode": "EventSemaphore",
                            "sync_info": {"on_update": [], "on_wait": [wv]},
                        })
                out.append(ins)
            blk["instructions"] = out
    return data


def _patch_bass_class():
    import json as _json
    cls = bass.Bass
    if getattr(cls, "_waitfix_patched", False):
        return
    orig = cls.to_json_bytes

    def patched(self, *a, **kw):
        data = _json.loads(orig(self, *a, **kw))
        _fix_waits_json(data)
        return _json.dumps(data).encode()

    cls.to_json_bytes = patched
    cls._waitfix_patched = True


# ----------------------------------------------------------------------------
# Host-side constant fusion
# ----------------------------------------------------------------------------
def _make_consts(inp):
    qkv_w = inp['qkv_w'][:, :, 0, 0, 0].astype(np.float64)      # (192, 64)
    dw_mid = inp['dw_w'][:, 0, 1, :, :].astype(np.float64)      # (192, 3, 3)
    fc_w = inp['fc_w'][:, :, 0, 0, 0].astype(np.float64)        # (9, 24)
    fc_b = inp['fc_b'].astype(np.float32)
    dep_mid = inp['dep_w'][:, :, 1, :, :].astype(np.float64)    # (64, 9, 3, 3)
    dep_b = inp['dep_b'].astype(np.float32)
    ln_g = inp['ln_g'].astype(np.float64)
    ln_b = inp['ln_b'].astype(np.float64)
    A = -np.exp(inp['A_log'].astype(np.float64))                # (NG, DG, DSTATE)
    Wdt, bdt = inp['Wdt'].astype(np.float64), inp['bdt'].astype(np.float64)
    WB, WC = inp['WB'].astype(np.float64), inp['WC'].astype(np.float64)
    Dp = inp['Dp'].astype(np.float64)
    out_w, out_b = inp['out_w'].astype(np.float64), inp['out_b'].astype(np.float64)

    c = {}
    # conv branch: per-tap fused (72, 64) matrices
    FCbd = np.zeros((72, 192))
    for d in range(8):
        for o in range(9):
            for k in range(24):
                FCbd[d * 9 + o, k * 8 + d] = fc_w[o, k]
    wt = {}
    for ty in range(3):
        for tx in range(3):
            wt[(ty, tx)] = (FCbd @ (dw_mid[:, ty, tx][:, None] * qkv_w)).T
    # paired lhsT: rows 0-63 = tap (ty,0), rows 64-127 = tap (ty,1)
    wtapP = np.zeros((128, 3 * 72), np.float32)
    wtapS = np.zeros((64, 3 * 72), np.float32)
    for ty in range(3):
        wtapP[0:64, 72 * ty:72 * ty + 72] = wt[(ty, 0)]
        wtapP[64:128, 72 * ty:72 * ty + 72] = wt[(ty, 1)]
        wtapS[:, 72 * ty:72 * ty + 72] = wt[(ty, 2)]
    c['wtapP'] = wtapP
    c['wtapS'] = wtapS
    f2b = np.zeros((72, 1), np.float32)
    for d in range(8):
        for o in range(9):
            f2b[d * 9 + o, 0] = fc_b[o]
    c['f2_bias'] = f2b
    bdep = np.zeros((72, 9 * 64), np.float32)
    for ty in range(3):
        for tx in range(3):
            k = ty * 3 + tx
            Bt = np.zeros((64, 72))
            for g in range(8):
                Bt[8 * g:8 * g + 8, 9 * g:9 * g + 9] = dep_mid[8 * g:8 * g + 8, :, ty, tx]
            bdep[:, 64 * k:64 * k + 64] = Bt.T.astype(np.float32)
    c['bdep'] = bdep
    c['depb_pp'] = np.tile(dep_b, IPC).reshape(128, 1)

    # mamba projections: gamma folded into lhsT; rhs is the normalized xn so
    # no rank-1 mean corrections are needed anywhere.
    dtl = np.zeros((64, 64))
    bcl = np.zeros((64, 64))
    bdt_c = np.zeros(64)
    fbc_c = np.zeros(64)
    for g in range(NG):
        rows = slice(g * DG, (g + 1) * DG)
        gam = ln_g[rows][:, None]
        bet = ln_b[rows]
        dtl[rows, g * DG:(g + 1) * DG] = Wdt[g] * gam
        bcl[rows, g * 8:g * 8 + 8] = WB[g] * gam
        bcl[rows, 32 + g * 8:32 + g * 8 + 8] = WC[g] * gam
        bdt_c[g * DG:(g + 1) * DG] = bdt[g] + Wdt[g].T @ bet
        fbc_c[g * 8:g * 8 + 8] = WB[g].T @ bet
        fbc_c[32 + g * 8:32 + g * 8 + 8] = WC[g].T @ bet

    def blockdiag2(m):
        o = np.zeros((128, 128))
        o[0:64, 0:64] = m
        o[64:128, 64:128] = m
        return o

    # split per scan-direction class: A-groups (0,1) project from raster xn,
    # B-groups (2,3) from the transposed xn_T; outputs accumulate into one
    # PSUM tile (disjoint non-zero rows)
    selA = np.zeros(64)
    selA[0:2 * DG] = 1.0
    c['dtA_lhsT'] = blockdiag2(dtl * selA[None, :]).astype(np.float32)
    c['dtB_lhsT'] = blockdiag2(dtl * (1 - selA)[None, :]).astype(np.float32)
    selAbc = np.zeros(64)
    selAbc[0:16] = 1.0
    selAbc[32:48] = 1.0
    c['bcA_lhsT'] = blockdiag2(bcl * selAbc[None, :]).astype(np.float32)
    c['bcB_lhsT'] = blockdiag2(bcl * (1 - selAbc)[None, :]).astype(np.float32)
    c['bdt_pp'] = np.tile(bdt_c, IPC).reshape(128, 1).astype(np.float32)
    c['fbc_pp'] = np.tile(fbc_c, IPC).reshape(128, 1).astype(np.float32)
    # scan lane = n*16 + d (n-major: B/C broadcasts stay direct SBUF->SBUF)
    app = np.zeros((128, NG), np.float32)
    for g in range(NG):
        for n in range(DSTATE):
            for d in range(DG):
                app[n * 16 + d, g] = A[g, d, n]
    c['a_pp'] = app
    opl = np.zeros((128, NG * 64))
    for g in range(NG):
        for n in range(DSTATE):
            for d in range(DG):
                opl[n * 16 + d, g * 64:(g + 1) * 64] = out_w[:, g * DG + d]
    c['outproj_lhsT'] = opl.astype(np.float32)
    # Dp skip term folded with out_w: y += (out_w*Dp*gam) @ xn
    dpg = Dp.reshape(-1) * ln_g
    dpf = out_w * dpg[None, :]
    c['dpx_lhsT'] = blockdiag2(dpf.T).astype(np.float32)
    outb_eff = out_b + out_w @ (Dp.reshape(-1) * ln_b)
    c['outb_pp'] = np.tile(outb_eff, IPC).reshape(128, 1).astype(np.float32)
    c['gam_pp'] = np.tile(ln_g, IPC).reshape(128, 1).astype(np.float32)
    c['beta_pp'] = np.tile(ln_b, IPC).reshape(128, 1).astype(np.float32)
    c['ones128'] = np.ones((128, 1), np.float32)
    c['eps_pp'] = np.full((128, 1), 1e-5, np.float32)
    c['ca1_lhsT'] = (inp['ca_w1'].T / L).astype(np.float32)       # fold 1/L mean
    c['ca1_b'] = inp['ca_b1'].reshape(16, 1).astype(np.float32)
    c['ca2_lhsT'] = inp['ca_w2'].T.astype(np.float32)
    c['ca2bn_pp'] = -np.tile(inp['ca_b2'], IPC).reshape(128, 1).astype(np.float32)
    sl = np.zeros((128, 2), np.float32)
    sl[0:64, 0] = 1.0
    sl[64:128, 1] = 1.0
    c['stats_lhsT'] = sl
    return c


CONST_SPECS = [
    ('wtapP', [128, 3 * 72], BF16), ('wtapS', [64, 3 * 72], BF16),
    ('f2_bias', [72, 1], F32),
    ('bdep', [72, 9 * 64], BF16), ('depb_pp', [128, 1], F32),
    ('dtA_lhsT', [128, 128], BF16), ('dtB_lhsT', [128, 128], BF16),
    ('bcA_lhsT', [128, 128], BF16), ('bcB_lhsT', [128, 128], BF16),
    ('bdt_pp', [128, 1], F32), ('fbc_pp', [128, 1], F32),
    ('a_pp', [128, NG], F32), ('outproj_lhsT', [128, NG * 64], BF16),
    ('dpx_lhsT', [128, 128], BF16),
    ('outb_pp', [128, 1], F32), ('gam_pp', [128, 1], F32),
    ('beta_pp', [128, 1], F32), ('ones128', [128, 1], F32),
    ('eps_pp', [128, 1], F32),
    ('ca1_lhsT', [64, 16], BF16), ('ca1_b', [16, 1], F32),
    ('ca2_lhsT', [16, 64], BF16), ('ca2bn_pp', [128, 1], F32),
    ('stats_lhsT', [128, 2], BF16),
]


def _build(reps=1, has_beta=False):
    _patch_bass_class()
    nc = bass.Bass("TRN2")
    xin = nc.declare_dram_parameter("x", [IPC, C, H, W], F32, isOutput=False)
    out = nc.declare_dram_parameter("out", [IPC, C, H, W], F32, isOutput=True)
    dram = {n: nc.declare_dram_parameter(n, s, F32, isOutput=False)
            for n, s, _ in CONST_SPECS}

    xin_f = xin.rearrange("i c h w -> (i c) (h w)")
    out_f = out.rearrange("i c h w -> (i c) (h w)")

    with TileContext(nc) as tc:
        with tc.tile_pool(name="const", bufs=1) as kpool, \
             tc.tile_pool(name="pers", bufs=1) as pp, \
             tc.tile_pool(name="work", bufs=2) as wp, \
             tc.tile_pool(name="dram", bufs=1, space="DRAM") as dmp:

            kt = {}
            for name, shape, dt in CONST_SPECS:
                kt[name] = kpool.tile(shape, dt, tag=name, name=name)
                eng = nc.gpsimd if dt == BF16 else nc.sync
                eng.dma_start(kt[name][:], dram[name][:])

            for _rep in range(reps):
              # Phase A: input, stats/LN, projections, conv
              with tc.tile_pool(name=f"psA{_rep}", bufs=2, space="PSUM") as psA, \
                   tc.tile_pool(name=f"psB{_rep}", bufs=2, space="PSUM") as psB, \
                   tc.tile_pool(name=f"psC{_rep}", bufs=2, space="PSUM") as psC:
                # ---- input load + padded tiles (0-63 plain, 64-127 one col
                # left-shifted, for f2 tap pairing) ----
                xraw = pp.tile([128, L], BF16, tag="xraw")
                nc.gpsimd.dma_start(xraw[:], xin_f[:])
                x_pad = []
                for i in range(IPC):
                    t = pp.tile([128, PADL], BF16, tag=f"x_pad{i}")
                    xpv = t[:, :].rearrange("c (h w) -> c h w", h=Hp)
                    # border-only zeros: top half holds the image at cols
                    # 1..64, bottom (shifted) at cols 0..63. Interior writes
                    # never touch the borders and the tile persists across
                    # reps, so the memsets run once.
                    if _rep == 0:
                        nc.vector.memset(xpv[:, 0:1, :], 0.0)
                        nc.vector.memset(xpv[:, Hp - 1:Hp, :], 0.0)
                        nc.vector.memset(xpv[:, 1:Hp - 1, Wp - 1:Wp], 0.0)
                        nc.vector.memset(xpv[0:64, 1:Hp - 1, 0:1], 0.0)
                        nc.vector.memset(xpv[64:128, 1:Hp - 1, W:W + 1], 0.0)
                    src = xraw[i * 64:(i + 1) * 64, :] \
                        .rearrange("c (h w) -> c h w", h=H)
                    if i == 0:
                        nc.vector.tensor_scalar_mul(
                            xpv[0:64, 1:H + 1, 1:W + 1], src, 1.0)
                        nc.sync.dma_start(xpv[64:128, 1:H + 1, 0:W], src)
                    else:
                        nc.sync.dma_start(xpv[0:64, 1:H + 1, 1:W + 1], src)
                        nc.vector.tensor_scalar_mul(
                            xpv[64:128, 1:H + 1, 0:W], src, 1.0)
                    x_pad.append(t)

                f2_pad = []
                for i in range(IPC):
                    t = pp.tile([72, PADL], BF16, tag=f"f2_pad{i}")
                    fv = t[:, :].rearrange("c (h w) -> c h w", h=Hp)
                    if _rep == 0:
                        nc.vector.memset(fv[:, 0:1, :], 0.0)
                        nc.vector.memset(fv[:, Hp - 1:Hp, :], 0.0)
                        nc.vector.memset(fv[:, 1:Hp - 1, 0:1], 0.0)
                        nc.vector.memset(fv[:, 1:Hp - 1, Wp - 1:Wp], 0.0)
                    f2_pad.append(t)

                # ---- stats in row form: s1 and s2 in separate base-0 tiles
                # (two-SBUF-input ops require equal base partitions) ----
                s_rows = pp.tile([2, L], BF16, tag="s_rows")
                s2_rows = pp.tile([2, L], BF16, tag="s2_rows")
                for cb in range(NCH):
                    cs = slice(cb * TC, (cb + 1) * TC)
                    sq = wp.tile([128, TC], BF16, tag="sq", bufs=2)
                    nc.scalar.activation(sq[:], xraw[:, cs], AF.Square)
                    s1p = psA.tile([2, TC], F32, tag="psA")
                    s2p = psA.tile([2, TC], F32, tag="psA")
                    nc.tensor.matmul(s1p[:], kt['stats_lhsT'][:],
                                     xraw[:, cs], start=True, stop=True)
                    nc.tensor.matmul(s2p[:], kt['stats_lhsT'][:], sq[:],
                                     start=True, stop=True)
                    nc.scalar.activation(s_rows[:, cs], s1p[:], AF.Copy)
                    nc.scalar.activation(s2_rows[:, cs], s2p[:], AF.Copy)
                # row-form LN math (no transposes):
                #   vm = (s1/64)^2 ; vm = s2/64 - vm (= var)
                #   rstd = exp(-0.5*ln(vm+1e-5)) ; vm = (s1/64)*rstd (= mur)
                vm = pp.tile([2, L], BF16, tag="vm")
                nc.scalar.activation(vm[:], s_rows[0:2, :], AF.Square,
                                     scale=1.0 / 64)
                nc.vector.scalar_tensor_tensor(vm[:], s2_rows[:, :],
                                               1.0 / 64, vm[:],
                                               OP.mult, OP.subtract)
                nc.scalar.activation(vm[:], vm[:], AF.Ln,
                                     bias=kt['eps_pp'][0:2, :])
                r_rstd = pp.tile([2, L], BF16, tag="r_rstd")
                nc.scalar.activation(r_rstd[:], vm[:], AF.Exp, scale=-0.5)
                nc.vector.scalar_tensor_tensor(vm[:], s_rows[0:2, :],
                                               1.0 / 64, r_rstd[:],
                                               OP.mult, OP.mult)
                # big broadcast tiles live in the work pool, sharing the scan
                # phase's b_rep/c_rep buffers (dead before those are written)
                rstd_bc = wp.tile([128, L], BF16, tag="b_rep",
                                  name="rstd_bc", bufs=2)
                nc.sync.dma_start(
                    rstd_bc[:],
                    r_rstd[:, :].unsqueeze(1).broadcast_to([2, 64, L]))
                mur_bc = wp.tile([128, L], BF16, tag="c_rep",
                                 name="mur_bc", bufs=2)
                nc.sync.dma_start(
                    mur_bc[:],
                    vm[:, :].unsqueeze(1).broadcast_to([2, 64, L]))

                # xn = x*rstd - mur  (normalized x, gamma/beta folded later)
                xn = pp.tile([128, L], BF16, tag="xn")
                nc.vector.tensor_mul(xn[:], xraw[:], rstd_bc[:])
                nc.vector.tensor_sub(xn[:], xn[:], mur_bc[:])
                # xn_T: free-dim 64x64 transpose of xn (one strided DVE pass);
                # feeds the column-major-scanned groups 2,3. Shares the scan
                # phase's h_sb buffer (dead before the first scan runs).
                xn_T = wp.tile([128, L], BF16, tag="h_sb", name="xn_T",
                               bufs=1)
                nc.vector.tensor_scalar_mul(
                    xn_T[:, :].rearrange("p (x y) -> p x y", x=W),
                    xn[:, :].rearrange("p (y x) -> p x y", y=H), 1.0)

                # ---- dt / B / C projections: groups 0,1 from xn, groups 2,3
                # from xn_T, accumulated into one PSUM tile ----
                dtu = pp.tile([128, 2 * L], BF16, tag="dtu")
                dt_sb = dtu[:, 0:L]
                u_sb = dtu[:, L:2 * L]
                bc_sb = pp.tile([128, L], BF16, tag="bc_sb")
                for cb in range(NCH):
                    cs = slice(cb * TC, (cb + 1) * TC)
                    dtp = psB.tile([128, TC], F32, tag="psB")
                    bcp = psB.tile([128, TC], F32, tag="psB")
                    nc.tensor.matmul(dtp[:], kt['dtA_lhsT'][:], xn[:, cs],
                                     start=True, stop=False)
                    nc.tensor.matmul(dtp[:], kt['dtB_lhsT'][:], xn_T[:, cs],
                                     start=False, stop=True)
                    nc.tensor.matmul(bcp[:], kt['bcA_lhsT'][:], xn[:, cs],
                                     start=True, stop=False)
                    nc.tensor.matmul(bcp[:], kt['bcB_lhsT'][:], xn_T[:, cs],
                                     start=False, stop=True)
                    # softplus part 1: exp(z + bdt), straight from PSUM
                    nc.scalar.activation(dtu[:, cs], dtp[:], AF.Exp,
                                         bias=kt['bdt_pp'][:])
                    if has_beta:
                        nc.scalar.activation(bc_sb[:, cs], bcp[:], AF.Identity,
                                             bias=kt['fbc_pp'][:])
                    else:
                        nc.scalar.activation(bc_sb[:, cs], bcp[:], AF.Copy)
                # softplus part 2: dt = ln(1 + exp(...))
                nc.scalar.activation(dt_sb[:, :], dt_sb[:, :], AF.Ln,
                                     bias=kt['ones128'][:])

                # u = dt * (gam*xn); groups 2,3 read xn_T (quadrant slices)
                for i in range(IPC):
                    qa = slice(i * 64, i * 64 + 32)
                    qb = slice(i * 64 + 32, i * 64 + 64)
                    nc.vector.scalar_tensor_tensor(
                        u_sb[qa, :], dt_sb[qa, :], kt['gam_pp'][qa, :],
                        xn[qa, :], OP.mult, OP.mult)
                    nc.vector.scalar_tensor_tensor(
                        u_sb[qb, :], dt_sb[qb, :], kt['gam_pp'][qb, :],
                        xn_T[qb, :], OP.mult, OP.mult)
                if has_beta:
                    nc.vector.scalar_tensor_tensor(u_sb[:], dt_sb[:],
                                                   kt['beta_pp'][:], u_sb[:],
                                                   OP.mult, OP.add)
                # DRAM bounce: SBUF sources cannot lead with a stride-0
                # repeat, so the (d -> n*16+d) replication reads from DRAM.
                # Split halves: dt uploads (SP ring) while u is still being
                # computed; u follows on the ACT ring.
                dtu_dram = dmp.tile([128, 2 * L], BF16, tag="dtu_dram",
                                    name=f"dtu_dram_{_rep}")
                nc.sync.dma_start(dtu_dram[:, 0:L], dt_sb)
                nc.scalar.dma_start(dtu_dram[:, L:2 * L], u_sb)

                # ---- conv branch ----
                def f2_unit(i, cb):
                    xv = x_pad[i][:, :].rearrange("c (h w) -> c h w", h=Hp)
                    f2v = f2_pad[i][:, :].rearrange("c (h w) -> c h w", h=Hp)
                    fp = psC.tile([72, TC], F32, tag="psC")
                    for ty in range(3):
                        # pair: taps (ty,0)+(ty,1) via the shifted bottom half
                        nc.tensor.matmul(
                            fp[:], kt['wtapP'][:, 72 * ty:72 * ty + 72],
                            xv[:, 8 * cb + ty: 8 * cb + ty + 8, 0:64],
                            start=(ty == 0), stop=False,
                            skip_group_check=True)
                        # single: tap (ty,2)
                        nc.tensor.matmul(
                            fp[:], kt['wtapS'][:, 72 * ty:72 * ty + 72],
                            xv[0:64, 8 * cb + ty: 8 * cb + ty + 8, 2:66],
                            start=False, stop=(ty == 2),
                            skip_group_check=True)
                    nc.scalar.activation(
                        f2v[:, 8 * cb + 1: 8 * cb + 9, 1: W + 1],
                        fp[:].rearrange("c (a b) -> c a b", a=8),
                        AF.Identity, bias=kt['f2_bias'][:])

                oc_sb = pp.tile([128, L], BF16, tag="oc_sb")

                def dep_unit(cb):
                    op_ps = psC.tile([128, TC], F32, tag="psC")
                    for i in range(IPC):
                        f2v = f2_pad[i][:, :].rearrange("c (h w) -> c h w", h=Hp)
                        for ty in range(3):
                            for tx in range(3):
                                k = ty * 3 + tx
                                nc.tensor.matmul(
                                    op_ps[i * 64:(i + 1) * 64, :],
                                    kt['bdep'][:, 64 * k:64 * k + 64],
                                    f2v[:, 8 * cb + ty: 8 * cb + ty + 8, tx: tx + 64],
                                    start=(k == 0), stop=(k == 8),
                                    tile_position=(0, i * 64),
                                    skip_group_check=True)
                    nc.scalar.activation(oc_sb[:, cb * TC:(cb + 1) * TC],
                                         op_ps[:], AF.Copy)

                if 'conv' not in DEBUG_SKIP:
                    for cb in range(NCH):
                        f2_unit(0, cb)
                        f2_unit(1, cb)
                    for cb in range(NCH):
                        dep_unit(cb)
                else:
                    nc.vector.memset(oc_sb[:], 0.0)

              # Phase B: selective scan per (image, group), lanes (d*8+n);
              # scan directions are pure APs; out-projection accumulates into
              # 8 resident y banks
              with tc.tile_pool(name=f"psY{_rep}", bufs=1, space="PSUM") as psY:
                y_ps = [psY.tile([128, TC], F32, tag=f"yc{cb}",
                                 name=f"yc{cb}_{_rep}") for cb in range(NCH)]
                first = [[True, True] for _ in range(NCH)]
                igs = [(i, g) for i in range(IPC) for g in range(NG)]

                def emit_bcasts(k):
                    i, g = igs[k]
                    drs = slice(i * 64 + g * 16, i * 64 + (g + 1) * 16)
                    brs = slice(i * 64 + g * 8, i * 64 + g * 8 + 8)
                    crs = slice(i * 64 + 32 + g * 8, i * 64 + 32 + g * 8 + 8)
                    # lane (n*16+d) <- dt/u row d via the DRAM copy (DRAM src
                    # APs have no partition-step constraint: 8x repeat leads).
                    # Alternate the two HWDGE rings so the 2MB broadcasts of
                    # consecutive units don't queue FIFO on one ring.
                    eng_a = nc.sync if k % 2 == 0 else nc.scalar
                    eng_b = nc.scalar if k % 2 == 0 else nc.sync
                    du_rep = wp.tile([128, 2 * L], BF16, tag="du_rep",
                                     name=f"du_rep{k}_{_rep}", bufs=2)
                    eng_a.dma_start(
                        du_rep[:],
                        dtu_dram[drs, :].unsqueeze(0).broadcast_to([8, 16, 2 * L]))
                    # lane (n*16+d) <- B/C row n: direct SBUF->SBUF
                    b_rep = wp.tile([128, L], BF16, tag="b_rep",
                                    name=f"b_rep{k}_{_rep}", bufs=2)
                    eng_b.dma_start(
                        b_rep[:],
                        bc_sb[brs, :].unsqueeze(1).broadcast_to([8, 16, L]))
                    c_rep = wp.tile([128, L], BF16, tag="c_rep",
                                    name=f"c_rep{k}_{_rep}", bufs=2)
                    nc.gpsimd.dma_start(
                        c_rep[:],
                        bc_sb[crs, :].unsqueeze(1).broadcast_to([8, 16, L]))
                    return du_rep, b_rep, c_rep

                pend = None if 'scan' in DEBUG_SKIP else emit_bcasts(0)
                for k, (i, g) in enumerate(igs if 'scan' not in DEBUG_SKIP else []):
                    du_rep, b_rep, c_rep = pend
                    if k + 1 < len(igs):
                        pend = emit_bcasts(k + 1)
                    dA = du_rep[:, 0:L]
                    nc.scalar.activation(dA, dA, AF.Exp,
                                         scale=kt['a_pp'][:, g:g + 1])
                    nc.vector.tensor_mul(b_rep[:], du_rep[:, L:2 * L],
                                         b_rep[:])
                    h_sb = wp.tile([128, L], BF16, tag="h_sb",
                                   name=f"h_sb{k}_{_rep}", bufs=1)
                    # groups 2,3 hold column-major data, so every scan is a
                    # contiguous run; odd groups reverse via step -1
                    if g % 2 == 0:
                        nc.vector.tensor_tensor_scan(
                            h_sb[:], dA, b_rep[:], 0.0, OP.mult, OP.add)
                    else:
                        nc.vector.tensor_tensor_scan(
                            h_sb[:, ::-1], dA[:, ::-1], b_rep[:, ::-1],
                            0.0, OP.mult, OP.add)
                    # zc = h * C. For the column-major groups, write zc back
                    # to raster order via a transposed dest AP, into the dead
                    # dA half of du_rep (scratch), so outproj rhs stays
                    # contiguous.
                    if g < 2:
                        nc.vector.tensor_mul(c_rep[:], h_sb[:], c_rep[:])
                        zc = c_rep
                    else:
                        nc.vector.tensor_tensor(
                            du_rep[:, 0:L].rearrange("p (y x) -> p x y", y=H),
                            h_sb[:, :].rearrange("p (x y) -> p x y", x=W),
                            c_rep[:, :].rearrange("p (x y) -> p x y", x=W),
                            OP.mult)
                        zc = du_rep
                    if 'outproj' not in DEBUG_SKIP:
                        for cb in range(NCH):
                            cs = slice(cb * TC, (cb + 1) * TC)
                            nc.tensor.matmul(
                                y_ps[cb][i * 64:(i + 1) * 64, :],
                                kt['outproj_lhsT'][:, g * 64:(g + 1) * 64],
                                zc[:, cs], start=first[cb][i], stop=False,
                                tile_position=(0, i * 64), skip_group_check=True)
                            first[cb][i] = False

                # Dp skip term (folded with out_w, rhs = xn), then close y
                y_sb = pp.tile([128, L], BF16, tag="y_sb")
                ymean = pp.tile([128, NCH], F32, tag="ymean")
                for cb in range(NCH):
                    cs = slice(cb * TC, (cb + 1) * TC)
                    nc.tensor.matmul(y_ps[cb][:], kt['dpx_lhsT'][:], xn[:, cs],
                                     start=('outproj' in DEBUG_SKIP
                                            or 'scan' in DEBUG_SKIP),
                                     stop=True, skip_group_check=True)
                    nc.scalar.activation(y_sb[:, cs], y_ps[cb][:], AF.Identity,
                                         bias=kt['outb_pp'][:],
                                         accum_out=ymean[:, cb:cb + 1])

              # Phase C: CA gate + final combine
              with tc.tile_pool(name=f"psZ{_rep}", bufs=2, space="PSUM") as psZ:
                ymv = wp.tile([128, 1], F32, tag="ymv")
                nc.vector.tensor_reduce(ymv[:], ymean[:], mybir.AxisListType.X, OP.add)
                ymc = []
                for i in range(IPC):
                    t = wp.tile([64, 1], BF16, tag=f"ymc{i}")
                    nc.gpsimd.dma_start(t[:], ymv[i * 64:(i + 1) * 64, :])
                    ymc.append(t)
                ca1 = psZ.tile([16, IPC], F32, tag="psZ")
                for i in range(IPC):
                    nc.tensor.matmul(ca1[:, i:i + 1], kt['ca1_lhsT'][:], ymc[i][:],
                                     start=True, stop=True)
                ca1s = wp.tile([16, IPC], BF16, tag="ca1s")
                nc.scalar.activation(ca1s[:], ca1[:], AF.Relu, bias=kt['ca1_b'][:])
                ca2 = psZ.tile([128, 1], F32, tag="psZ")
                for i in range(IPC):
                    nc.tensor.matmul(ca2[i * 64:(i + 1) * 64, :], kt['ca2_lhsT'][:],
                                     ca1s[:, i:i + 1], start=True, stop=True,
                                     tile_position=(0, i * 64),
                                     skip_group_check=True)
                ca_sb = pp.tile([128, 1], F32, tag="ca_sb")
                nc.scalar.activation(ca_sb[:], ca2[:], AF.Exp, scale=-1.0,
                                     bias=kt['ca2bn_pp'][:])
                nc.vector.tensor_scalar_add(ca_sb[:], ca_sb[:], 1.0)
                nc.vector.reciprocal(ca_sb[:], ca_sb[:])

                # ---- final combine: out = x + oc + depb + ca*y ----
                res = pp.tile([128, L], BF16, tag="xn", name="res")
                nc.vector.scalar_tensor_tensor(res[:], oc_sb[:], kt['depb_pp'][:],
                                               xraw[:], OP.add, OP.add)
                nc.vector.scalar_tensor_tensor(res[:], y_sb[:], ca_sb[:],
                                               res[:], OP.mult, OP.add)
                nc.gpsimd.dma_start(out_f[:], res[:])

    return nc


def _make_runner(nc):
    """Compile nc once into a cached PJRT executable over the 8 cores."""
    import jax
    from jax.sharding import Mesh, PartitionSpec
    from jax.experimental.shard_map import shard_map
    from concourse import bass2jax

    bass2jax.install_neuronx_cc_hook()
    partition_name = nc.partition_id_tensor.name if nc.partition_id_tensor else None
    in_names, out_names, out_avals, zero_shapes = [], [], [], []
    for alloc in nc.m.functions[0].allocations:
        if not isinstance(alloc, mybir.MemoryLocationSet):
            continue
        name = alloc.memorylocations[0].name
        if alloc.kind == "ExternalInput":
            if name != partition_name:
                in_names.append(name)
        elif alloc.kind == "ExternalOutput":
            out_names.append(name)
            shape = tuple(alloc.tensor_shape)
            dtype = mybir.dt.np(alloc.dtype)
            out_avals.append(jax.core.ShapedArray(shape, dtype))
            zero_shapes.append((shape, dtype))
    n_params = len(in_names)
    n_outs = len(out_avals)
    in_names.extend(out_names)
    if partition_name is not None:
        in_names.append(partition_name)

    def _body(*args):
        operands = list(args)
        if partition_name is not None:
            operands.append(bass2jax.partition_id_tensor())
        outs = bass2jax._bass_exec_p.bind(
            *operands, out_avals=tuple(out_avals), in_names=tuple(in_names),
            out_names=tuple(out_names), lowering_input_output_aliases=(),
            sim_require_finite=True, sim_require_nnan=True, nc=nc)
        return tuple(outs)

    devices = jax.devices()[:NCORES]
    mesh = Mesh(np.asarray(devices), ("core",))
    in_specs = (PartitionSpec("core"),) * (n_params + n_outs)
    out_specs = (PartitionSpec("core"),) * len(out_names)
    donate = tuple(range(n_params, n_params + n_outs))
    sharded = jax.jit(
        shard_map(_body, mesh=mesh, in_specs=in_specs, out_specs=out_specs,
                  check_rep=False),
        donate_argnums=donate, keep_unused=True)

    from jax.sharding import NamedSharding
    zero_fns = [
        jax.jit(lambda s=s, d=d: jax.numpy.zeros((NCORES * s[0], *s[1:]), d),
                out_shardings=NamedSharding(mesh, PartitionSpec("core")))
        for s, d in zero_shapes]

    import hashlib
    dev_cache = {}

    def _to_dev(arr):
        key = hashlib.blake2b(arr.tobytes(), digest_size=16).digest()
        hit = dev_cache.get(key)
        if hit is None:
            hit = jax.device_put(
                arr, NamedSharding(mesh, PartitionSpec("core")))
            jax.block_until_ready(hit)
            dev_cache[key] = hit
        return hit

    def run(in_maps):
        per_core = [[np.asarray(m[nm]) for nm in in_names[:n_params]]
                    for m in in_maps]
        concat_in = [
            _to_dev(np.ascontiguousarray(np.concatenate(
                [per_core[c][i] for c in range(NCORES)], axis=0)))
            for i in range(n_params)]
        concat_zeros = [fn() for fn in zero_fns]
        out_arrs = sharded(*concat_in, *concat_zeros)
        return [
            {name: np.asarray(out_arrs[i]).reshape(NCORES, *out_avals[i].shape)[c]
             for i, name in enumerate(out_names)}
            for c in range(NCORES)]

    return run


def kernel(__reps=1, **inputs):
    inputs = {k: np.asarray(v) for k, v in inputs.items()}
    x = inputs['x'].astype(np.float32)
    has_beta = bool(np.any(inputs['ln_b'] != 0))
    key = f"v3r{__reps}b{int(has_beta)}"
    consts = _make_consts(inputs)
    in_maps = []
    for core in range(NCORES):
        m = {'x': np.ascontiguousarray(x[core * IPC:(core + 1) * IPC])}
        for name, _, _ in CONST_SPECS:
            m[name] = np.ascontiguousarray(consts[name].astype(np.float32))
        in_maps.append(m)
    if key not in _CACHE:
        nc = _build(__reps, has_beta)
        try:
            _CACHE[key] = ('runner', _make_runner(nc))
        except Exception:
            _CACHE[key] = ('nc', nc)
    kind, obj = _CACHE[key]
    if kind == 'runner':
        results = obj(in_maps)
        outs = [results[i]['out'] for i in range(NCORES)]
    else:
        res = run_bass_kernel_spmd(obj, in_maps, list(range(NCORES)))
        outs = [res.results[i]['out'] for i in range(NCORES)]
    return np.concatenate(outs, axis=0).astype(np.float32)
